# revision 4
# baseline (speedup 1.0000x reference)
"""Causal multi-head attention block on 8 trn2 NeuronCores.

Problem: B=2, S=2048, D=768, H=12, Dh=64 (fp32), causal softmax attention
with QKV projections and output projection summed over heads.

Sharding: tensor-parallel over heads x data-parallel over batch.
core c in [0,8): b = c//4, heads = {3g, 3g+1, 3g+2} with g = c%4.
Each core computes the partial output sum over its 3 heads for its batch;
the host sums the 4 partials per batch (the TP all-reduce) and stacks.

Per-core device kernel (SPMD, identical program):
  - x[b]^T fed as [768, 2048] fp32 (layout choice of the shard), cast to
    bf16 on device.
  - QK projections emit Q^T/K^T [64, 2048] per head (heads pair-stacked on
    partitions where possible); V kept [sk, 64] with a ones column appended
    so the z matmul also yields the softmax denominator D (row 64).
  - scores computed transposed: S^T[sk, sq] = K^T.T @ Q^T tiles, causal
    band only, exp on ScalarE (scale=1/8 folded in), triangular-diagonal
    masking via one [128,128] 0/1 mask multiply.
  - z^T[e, sq] accumulated over sk tiles; normalized by 1/D via a
    DMA-broadcast of the reciprocal row; output projection contracts the
    head pair jointly (K=128) plus the solo head (K=64) into shared PSUM.

Biases are all zeros per the problem spec (fill=zeros); b_O is applied on
the host if nonzero (exact). b_Q/b_K/b_V are asserted zero.
"""

import os
import sys
import types
import numpy as np

B, S, D, H, DH = 2, 2048, 768, 12, 64
N_CORES = 8
P = 128
NK = D // P      # 6 contraction chunks
NJ = S // 512    # 4 sq tiles of 512
NI = S // P      # 16 sk tiles of 128
SQT = 512

_PROGRAM = None
LAST_RESULTS = None


def _install_ntff_shim():
    """antenv.axon_hooks is missing in this image; shim it so trace=True works."""
    if "antenv.axon_hooks" in sys.modules:
        return
    try:
        from trn_agent_boot.trn_boot import _ntff_profile_via_ctypes
        m = types.ModuleType("antenv.axon_hooks")
        hook = _ntff_profile_via_ctypes("/opt/axon/libaxon_pjrt.so")
        m.get_axon_ntff_profile_hook = lambda: hook
        m.set_axon_ntff_profile_hook = lambda h: None
        sys.modules["antenv.axon_hooks"] = m
    except Exception:
        pass


def _build_program():
    import concourse.bass as bass
    import concourse.mybir as mybir
    from concourse import bacc
    from concourse.tile import TileContext
    from concourse.bass import ts, ds

    fp32 = mybir.dt.float32
    bf16 = mybir.dt.bfloat16
    Exp = mybir.ActivationFunctionType.Exp
    Mult = mybir.AluOpType.mult

    nc = bacc.Bacc("TRN2", target_bir_lowering=False, debug=False,
                   num_devices=N_CORES)

    xT = nc.dram_tensor("xT", (D, S), fp32, kind="ExternalInput")
    wq2 = nc.dram_tensor("wq2", (D, 128), fp32, kind="ExternalInput")
    wk2 = nc.dram_tensor("wk2", (D, 128), fp32, kind="ExternalInput")
    wqk3 = nc.dram_tensor("wqk3", (D, 128), fp32, kind="ExternalInput")
    wv = nc.dram_tensor("wv", (D, 192), fp32, kind="ExternalInput")
    wo2 = nc.dram_tensor("wo2", (128, D), fp32, kind="ExternalInput")
    wo3 = nc.dram_tensor("wo3", (DH, D), fp32, kind="ExternalInput")
    maskin = nc.dram_tensor("mask", (P, P), fp32, kind="ExternalInput")
    out = nc.dram_tensor("out", (S, D), fp32, kind="ExternalOutput")

    with TileContext(nc) as tc:
        with tc.tile_pool(name="work", bufs=1) as work, \
             tc.tile_pool(name="stage", bufs=2) as stage, \
             tc.tile_pool(name="epool", bufs=8) as epool, \
             tc.tile_pool(name="zsb", bufs=2) as zsb, \
             tc.tile_pool(name="zcol", bufs=2) as zcol, \
             tc.tile_pool(name="dram", bufs=2, space="DRAM") as dram, \
             tc.tile_pool(name="psum", bufs=2, space="PSUM") as psum:

            # ---------------- persistent SBUF tiles ----------------
            xTb = work.tile([P, NK, S], bf16, name="xTb")
            QT2 = work.tile([P, S], bf16, name="QT2")   # h0 rows 0:64, h1 rows 64:128
            KT2 = work.tile([P, S], bf16, name="KT2")
            QT3 = work.tile([64, S], bf16, name="QT3")
            KT3hi = work.tile([P, S], bf16, name="KT3hi")  # rows 64:128 = K3
            KT3 = work.tile([64, S], bf16, name="KT3")
            V0 = work.tile([P, NI, 65], bf16, name="V0")
            V1 = work.tile([P, NI, 65], bf16, name="V1")
            V2 = work.tile([P, NI, 65], bf16, name="V2")
            wq2b = work.tile([P, NK, 128], bf16, name="wq2b")
            wk2b = work.tile([P, NK, 128], bf16, name="wk2b")
            wqk3b = work.tile([P, NK, 128], bf16, name="wqk3b")
            wvb = work.tile([P, NK, 192], bf16, name="wvb")
            wo2b = work.tile([P, D], bf16, name="wo2b")
            wo3b = work.tile([64, D], bf16, name="wo3b")
            maskb = work.tile([P, P], bf16, name="maskb")

            # ---------------- load + cast weights ----------------
            def load_cast(dst, dram_ap, shape_f32):
                st = stage.tile(shape_f32, fp32, tag="wstage", name="wstage")
                nc.sync.dma_start(st[:], dram_ap)
                nc.vector.tensor_copy(dst[:], st[:])

            load_cast(wq2b, wq2[:].rearrange("(o p) e -> p o e", p=P), [P, NK, 128])
            load_cast(wk2b, wk2[:].rearrange("(o p) e -> p o e", p=P), [P, NK, 128])
            load_cast(wqk3b, wqk3[:].rearrange("(o p) e -> p o e", p=P), [P, NK, 128])
            load_cast(wvb, wv[:].rearrange("(o p) e -> p o e", p=P), [P, NK, 192])

            wo2st = stage.tile([P, D], fp32, tag="wostage", name="wo2st")
            nc.sync.dma_start(wo2st[:], wo2[:])
            nc.vector.tensor_copy(wo2b[:], wo2st[:])
            wo3st = stage.tile([64, D], fp32, tag="wostage", name="wo3st")
            nc.sync.dma_start(wo3st[:], wo3[:])
            nc.vector.tensor_copy(wo3b[:], wo3st[:])
            mst = stage.tile([P, P], fp32, tag="mstage", name="mst")
            nc.sync.dma_start(mst[:], maskin[:])
            nc.vector.tensor_copy(maskb[:], mst[:])

            # ---------------- load + cast x^T ----------------
            for k in range(NK):
                xst = stage.tile([P, S], fp32, tag="xstage", name="xst")
                nc.sync.dma_start(xst[:], xT[ts(k, P), :])
                nc.vector.tensor_copy(xTb[:, k, :], xst[:])

            # ---------------- QK projections ----------------
            # psum tags: "s" and "z" are shared with the attention phase.
            for st_i in range(NJ):
                sl = ts(st_i, SQT)
                q2p = psum.tile([P, SQT], fp32, tag="s", name="q2p")
                for k in range(NK):
                    nc.tensor.matmul(q2p[:], wq2b[:, k, :], xTb[:, k, sl],
                                     start=(k == 0), stop=(k == NK - 1))
                nc.vector.tensor_copy(QT2[:, sl], q2p[:])

                k2p = psum.tile([P, SQT], fp32, tag="s", name="k2p")
                for k in range(NK):
                    nc.tensor.matmul(k2p[:], wk2b[:, k, :], xTb[:, k, sl],
                                     start=(k == 0), stop=(k == NK - 1))
                nc.vector.tensor_copy(KT2[:, sl], k2p[:])

                qk3p = psum.tile([P, SQT], fp32, tag="s", name="qk3p")
                for k in range(NK):
                    nc.tensor.matmul(qk3p[:], wqk3b[:, k, :], xTb[:, k, sl],
                                     start=(k == 0), stop=(k == NK - 1))
                nc.vector.tensor_copy(QT3[:, sl], qk3p[0:64, :])
                nc.vector.tensor_copy(KT3hi[64:128, sl], qk3p[64:128, :])

            # K3 partition shift 64:128 -> 0:64 (DMA can cross partitions)
            nc.sync.dma_start(KT3[:, :], KT3hi[64:128, :])

            # ---------------- V projections ----------------
            for t in range(NI):
                vp = psum.tile([P, 192], fp32, tag="z", name="vp")
                for k in range(NK):
                    nc.tensor.matmul(vp[:], xTb[:, k, ts(t, P)], wvb[:, k, :],
                                     start=(k == 0), stop=(k == NK - 1))
                nc.vector.tensor_copy(V0[:, t, 0:64], vp[:, 0:64])
                nc.vector.tensor_copy(V1[:, t, 0:64], vp[:, 64:128])
                nc.vector.tensor_copy(V2[:, t, 0:64], vp[:, 128:192])
            nc.vector.memset(V0[:, :, 64], 1.0)
            nc.vector.memset(V1[:, :, 64], 1.0)
            nc.vector.memset(V2[:, :, 64], 1.0)

            # ---------------- attention ----------------
            heads = [
                (QT2[0:64, :], KT2[0:64, :], V0),
                (QT2[64:128, :], KT2[64:128, :], V1),
                (QT3[:, :], KT3[:, :], V2),
            ]

            for j in range(NJ):
                zT2 = zcol.tile([P, SQT], bf16, tag="zT2", name="zT2")
                zT3 = zcol.tile([64, SQT], bf16, tag="zT3", name="zT3")
                for hidx, (QTh, KTh, Vh) in enumerate(heads):
                    n_i = 4 * j + 4
                    z_ps = psum.tile([P, SQT], fp32, tag="z", name="z_ps")

                    def s_start(i):
                        col0 = P * (i - 4 * j) if i >= 4 * j else 0
                        s_ps = psum.tile([P, SQT], fp32, tag="s", name="s_ps")
                        nc.tensor.matmul(
                            s_ps[:, col0:SQT],
                            KTh[:, ts(i, P)],
                            QTh[:, ds(SQT * j + col0, SQT - col0)],
                            start=True, stop=True)
                        return s_ps, col0

                    pending = s_start(0)
                    for i in range(n_i):
                        s_ps, col0 = pending
                        E_t = epool.tile([P, SQT], bf16, name="E_t")
                        nc.scalar.activation(E_t[:, col0:SQT], s_ps[:, col0:SQT],
                                             Exp, scale=0.125)
                        if i >= 4 * j:
                            nc.vector.tensor_tensor(
                                E_t[:, col0:col0 + P], E_t[:, col0:col0 + P],
                                maskb[:], Mult)
                        if i + 1 < n_i:
                            pending = s_start(i + 1)
                        nc.tensor.matmul(
                            z_ps[0:65, col0:SQT],
                            Vh[:, i, :],
                            E_t[:, col0:SQT],
                            start=(i == 0), stop=(i == n_i - 1),
                            skip_group_check=True)

                    # normalize: zT = z[:64] * (1 / D) with D = z row 64
                    z_sb = zsb.tile([P, SQT], fp32, tag="z_sb", name="z_sb")
                    nc.vector.tensor_copy(z_sb[0:65, :], z_ps[0:65, :])
                    dinv = zsb.tile([1, SQT], fp32, tag="dinv", name="dinv")
                    nc.vector.reciprocal(dinv[:], z_sb[64:65, :])
                    dscr = dram.tile([1, SQT], fp32, name="dscr")
                    nc.sync.dma_start(dscr[:], dinv[:])
                    dbc = zsb.tile([64, SQT], fp32, tag="dbc", name="dbc")
                    nc.sync.dma_start(dbc[:], dscr[:].to_broadcast((64, SQT)))
                    if hidx == 0:
                        nc.vector.tensor_tensor(zT2[0:64, :], z_sb[0:64, :],
                                                dbc[:], Mult)
                    elif hidx == 1:
                        z1t = zcol.tile([64, SQT], bf16, tag="z1t", name="z1t")
                        nc.vector.tensor_tensor(z1t[:], z_sb[0:64, :],
                                                dbc[:], Mult)
                        nc.sync.dma_start(zT2[64:128, :], z1t[:])
                    else:
                        nc.vector.tensor_tensor(zT3[:], z_sb[0:64, :],
                                                dbc[:], Mult)

                # ---------------- output projection ----------------
                for c in range(4):
                    row = ds(SQT * j + P * c, P)
                    o1 = psum.tile([P, 512], fp32, tag="o1", name="o1")
                    nc.tensor.matmul(o1[:], zT2[:, ts(c, P)], wo2b[:, 0:512],
                                     start=True, stop=False,
                                     skip_group_check=True)
                    nc.tensor.matmul(o1[:], zT3[:, ts(c, P)], wo3b[:, 0:512],
                                     start=False, stop=True,
                                     skip_group_check=True)
                    o2 = psum.tile([P, 256], fp32, tag="o2", name="o2")
                    nc.tensor.matmul(o2[:], zT2[:, ts(c, P)], wo2b[:, 512:768],
                                     start=True, stop=False,
                                     skip_group_check=True)
                    nc.tensor.matmul(o2[:], zT3[:, ts(c, P)], wo3b[:, 512:768],
                                     start=False, stop=True,
                                     skip_group_check=True)
                    o1s = zsb.tile([P, 512], fp32, tag="o1s", name="o1s")
                    nc.vector.tensor_copy(o1s[:], o1[:])
                    o2s = zsb.tile([P, 256], fp32, tag="o2s", name="o2s")
                    nc.scalar.copy(o2s[:], o2[:])
                    nc.sync.dma_start(out[row, 0:512], o1s[:])
                    nc.sync.dma_start(out[row, 512:768], o2s[:])

    nc.compile()
    return nc


def _get_program():
    global _PROGRAM
    if _PROGRAM is None:
        _PROGRAM = _build_program()
    return _PROGRAM


def kernel(x, W_Q, W_K, W_V, W_O, b_Q, b_K, b_V, b_O):
    global LAST_RESULTS
    _install_ntff_shim()
    from concourse import bass_utils

    x = np.asarray(x, dtype=np.float32)
    W_Q = np.asarray(W_Q, dtype=np.float32)
    W_K = np.asarray(W_K, dtype=np.float32)
    W_V = np.asarray(W_V, dtype=np.float32)
    W_O = np.asarray(W_O, dtype=np.float32)
    b_Q = np.asarray(b_Q, dtype=np.float32)
    b_K = np.asarray(b_K, dtype=np.float32)
    b_V = np.asarray(b_V, dtype=np.float32)
    b_O = np.asarray(b_O, dtype=np.float32)
    assert not (np.any(b_Q) or np.any(b_K) or np.any(b_V)), \
        "kernel assumes zero QKV biases (problem spec fill=zeros)"

    nc = _get_program()

    mask = np.triu(np.ones((P, P), dtype=np.float32))
    xTs = [np.ascontiguousarray(x[b].T) for b in range(B)]

    in_maps = []
    for c in range(N_CORES):
        b, g = c // 4, c % 4
        hs = [3 * g, 3 * g + 1, 3 * g + 2]
        in_maps.append({
            "xT": xTs[b],
            "wq2": np.ascontiguousarray(
                np.concatenate([W_Q[hs[0]], W_Q[hs[1]]], axis=1)),
            "wk2": np.ascontiguousarray(
                np.concatenate([W_K[hs[0]], W_K[hs[1]]], axis=1)),
            "wqk3": np.ascontiguousarray(
                np.concatenate([W_Q[hs[2]], W_K[hs[2]]], axis=1)),
            "wv": np.ascontiguousarray(
                np.concatenate([W_V[hs[0]], W_V[hs[1]], W_V[hs[2]]], axis=1)),
            "wo2": np.ascontiguousarray(
                np.concatenate([W_O[hs[0]], W_O[hs[1]]], axis=0)),
            "wo3": np.ascontiguousarray(W_O[hs[2]]),
            "mask": mask,
        })

    res = bass_utils.run_bass_kernel_spmd(
        nc, in_maps, core_ids=list(range(N_CORES)),
        trace=bool(os.environ.get("BASS_TRACE")))
    LAST_RESULTS = res

    parts = [res.results[c]["out"] for c in range(N_CORES)]
    full = np.stack([
        parts[0] + parts[1] + parts[2] + parts[3],
        parts[4] + parts[5] + parts[6] + parts[7],
    ], axis=0)
    if np.any(b_O):
        full = full + b_O
    return full.astype(np.float32)


# revision 7
# speedup vs baseline: 1.0792x; 1.0792x over previous
"""Causal multi-head attention block on 8 trn2 NeuronCores.

Problem: B=2, S=2048, D=768, H=12, Dh=64 (fp32), causal softmax attention
with QKV projections and output projection summed over heads.

Sharding: tensor-parallel over heads x data-parallel over batch.
core c in [0,8): b = c//4, heads = {3g, 3g+1, 3g+2} with g = c%4.
Each core computes the partial output sum over its 3 heads for its batch;
the host sums the 4 partials per batch (the TP all-reduce) and stacks.

Per-core device kernel (SPMD, identical program):
  - x[b]^T fed as [768, 2048] fp32 (layout choice of the shard), cast to
    bf16 on device.
  - QK projections emit Q^T/K^T [64, 2048] per head (heads pair-stacked on
    partitions where possible); V kept [sk, 64] with a ones column appended
    so the z matmul also yields the softmax denominator D (row 64).
  - scores computed transposed: S^T[sk, sq] = K^T.T @ Q^T tiles, causal
    band only, exp on ScalarE (scale=1/8 folded in), triangular-diagonal
    masking via one [128,128] 0/1 mask multiply.
  - z^T[e, sq] accumulated over sk tiles; normalized by 1/D via a
    DMA-broadcast of the reciprocal row; output projection contracts the
    head pair jointly (K=128) plus the solo head (K=64) into shared PSUM.

Biases are all zeros per the problem spec (fill=zeros); b_O is applied on
the host if nonzero (exact). b_Q/b_K/b_V are asserted zero.
"""

import os
import sys
import types
import numpy as np

B, S, D, H, DH = 2, 2048, 768, 12, 64
N_CORES = 8
P = 128
NK = D // P      # 6 contraction chunks
NJ = S // 512    # 4 sq tiles of 512
NI = S // P      # 16 sk tiles of 128
SQT = 512

_PROGRAM = None
LAST_RESULTS = None


def _install_ntff_shim():
    """antenv.axon_hooks is missing in this image; shim it so trace=True works."""
    if "antenv.axon_hooks" in sys.modules:
        return
    try:
        from trn_agent_boot.trn_boot import _ntff_profile_via_ctypes
        m = types.ModuleType("antenv.axon_hooks")
        hook = _ntff_profile_via_ctypes("/opt/axon/libaxon_pjrt.so")
        m.get_axon_ntff_profile_hook = lambda: hook
        m.set_axon_ntff_profile_hook = lambda h: None
        sys.modules["antenv.axon_hooks"] = m
    except Exception:
        pass


def _build_program():
    import concourse.bass as bass
    import concourse.mybir as mybir
    from concourse import bacc
    from concourse.tile import TileContext
    from concourse.bass import ts, ds

    fp32 = mybir.dt.float32
    bf16 = mybir.dt.bfloat16
    Exp = mybir.ActivationFunctionType.Exp
    Mult = mybir.AluOpType.mult

    nc = bacc.Bacc("TRN2", target_bir_lowering=False, debug=False,
                   num_devices=N_CORES)

    xT = nc.dram_tensor("xT", (D, S), fp32, kind="ExternalInput")
    wq2 = nc.dram_tensor("wq2", (D, 128), fp32, kind="ExternalInput")
    wk2 = nc.dram_tensor("wk2", (D, 128), fp32, kind="ExternalInput")
    wqk3 = nc.dram_tensor("wqk3", (D, 128), fp32, kind="ExternalInput")
    wv = nc.dram_tensor("wv", (D, 192), fp32, kind="ExternalInput")
    wo2 = nc.dram_tensor("wo2", (128, D), fp32, kind="ExternalInput")
    wo3 = nc.dram_tensor("wo3", (DH, D), fp32, kind="ExternalInput")
    maskin = nc.dram_tensor("mask", (P, P), fp32, kind="ExternalInput")
    out = nc.dram_tensor("out", (S, D), fp32, kind="ExternalOutput")

    with TileContext(nc) as tc:
        with tc.tile_pool(name="work", bufs=1) as work, \
             tc.tile_pool(name="stage", bufs=2) as stage, \
             tc.tile_pool(name="epool", bufs=8) as epool, \
             tc.tile_pool(name="zsb", bufs=2) as zsb, \
             tc.tile_pool(name="zcol", bufs=2) as zcol, \
             tc.tile_pool(name="dram", bufs=2, space="DRAM") as dram, \
             tc.tile_pool(name="psum", bufs=2, space="PSUM") as psum:

            # ---------------- persistent SBUF tiles ----------------
            xTb = work.tile([P, NK, S], bf16, name="xTb")
            QT2 = work.tile([P, S], bf16, name="QT2")   # h0 rows 0:64, h1 rows 64:128
            KT2 = work.tile([P, S], bf16, name="KT2")
            QT3 = work.tile([64, S], bf16, name="QT3")
            KT3hi = work.tile([P, S], bf16, name="KT3hi")  # rows 64:128 = K3
            KT3 = work.tile([64, S], bf16, name="KT3")
            V_all = work.tile([P, NI, 3, 65], bf16, name="V_all")
            wq2b = work.tile([P, NK, 128], bf16, name="wq2b")
            wk2b = work.tile([P, NK, 128], bf16, name="wk2b")
            wqk3b = work.tile([P, NK, 128], bf16, name="wqk3b")
            wvb = work.tile([P, NK, 192], bf16, name="wvb")
            wo2b = work.tile([P, D], bf16, name="wo2b")
            wo3b = work.tile([64, D], bf16, name="wo3b")
            maskb = work.tile([P, P], bf16, name="maskb")

            # ---------------- load + cast weights ----------------
            def load_cast(dst, dram_ap, shape_f32):
                st = stage.tile(shape_f32, fp32, tag="wstage", name="wstage")
                nc.sync.dma_start(st[:], dram_ap)
                nc.vector.tensor_copy(dst[:], st[:])

            load_cast(wq2b, wq2[:].rearrange("(o p) e -> p o e", p=P), [P, NK, 128])
            load_cast(wk2b, wk2[:].rearrange("(o p) e -> p o e", p=P), [P, NK, 128])
            load_cast(wqk3b, wqk3[:].rearrange("(o p) e -> p o e", p=P), [P, NK, 128])
            load_cast(wvb, wv[:].rearrange("(o p) e -> p o e", p=P), [P, NK, 192])

            wo2st = stage.tile([P, D], fp32, tag="wostage", name="wo2st")
            nc.sync.dma_start(wo2st[:], wo2[:])
            nc.vector.tensor_copy(wo2b[:], wo2st[:])
            wo3st = stage.tile([64, D], fp32, tag="wostage", name="wo3st")
            nc.sync.dma_start(wo3st[:], wo3[:])
            nc.vector.tensor_copy(wo3b[:], wo3st[:])
            mst = stage.tile([P, P], fp32, tag="mstage", name="mst")
            nc.sync.dma_start(mst[:], maskin[:])
            nc.vector.tensor_copy(maskb[:], mst[:])

            # ---------------- load + cast x^T ----------------
            for k in range(NK):
                xst = stage.tile([P, S], fp32, tag="xstage", name="xst")
                nc.sync.dma_start(xst[:], xT[ts(k, P), :])
                nc.vector.tensor_copy(xTb[:, k, :], xst[:])

            # ---------------- QK projections ----------------
            # psum tags: "s" and "z" are shared with the attention phase.
            for st_i in range(NJ):
                sl = ts(st_i, SQT)
                q2p = psum.tile([P, SQT], fp32, tag="s", name="q2p")
                for k in range(NK):
                    nc.tensor.matmul(q2p[:], wq2b[:, k, :], xTb[:, k, sl],
                                     start=(k == 0), stop=(k == NK - 1))
                nc.scalar.copy(QT2[:, sl], q2p[:])

                k2p = psum.tile([P, SQT], fp32, tag="s", name="k2p")
                for k in range(NK):
                    nc.tensor.matmul(k2p[:], wk2b[:, k, :], xTb[:, k, sl],
                                     start=(k == 0), stop=(k == NK - 1))
                nc.scalar.copy(KT2[:, sl], k2p[:])

                qk3p = psum.tile([P, SQT], fp32, tag="s", name="qk3p")
                for k in range(NK):
                    nc.tensor.matmul(qk3p[:], wqk3b[:, k, :], xTb[:, k, sl],
                                     start=(k == 0), stop=(k == NK - 1))
                nc.scalar.copy(QT3[:, sl], qk3p[0:64, :])
                nc.scalar.copy(KT3hi[64:128, sl], qk3p[64:128, :])

            # K3 partition shift 64:128 -> 0:64 (DMA can cross partitions)
            nc.sync.dma_start(KT3[:, :], KT3hi[64:128, :])

            # ---------------- V projections ----------------
            for t in range(NI):
                vp = psum.tile([P, 192], fp32, tag="z", name="vp")
                for k in range(NK):
                    nc.tensor.matmul(vp[:], xTb[:, k, ts(t, P)], wvb[:, k, :],
                                     start=(k == 0), stop=(k == NK - 1))
                nc.scalar.copy(V_all[:, t, :, 0:64],
                               vp[:].rearrange("p (h e) -> p h e", h=3))
            nc.vector.memset(V_all[:, :, :, 64], 1.0)

            # ---------------- attention ----------------

            for j in range(NJ):
                zT2 = zcol.tile([P, SQT], bf16, tag="zT2", name="zT2")
                zT3 = zcol.tile([64, SQT], bf16, tag="zT3", name="zT3")
                n_i = 4 * j + 4

                def s_start(h, i):
                    QTh = QT3 if h == 2 else QT2[64 * h:64 * h + 64, :]
                    KTh = KT3 if h == 2 else KT2[64 * h:64 * h + 64, :]
                    col0 = P * (i - 4 * j) if i >= 4 * j else 0
                    s_ps = psum.tile([P, SQT], fp32, tag="s", name="s_ps")
                    nc.tensor.matmul(
                        s_ps[:, col0:SQT],
                        KTh[:, ts(i, P)],
                        QTh[:, ds(SQT * j + col0, SQT - col0)],
                        start=True, stop=True)
                    return s_ps, col0

                def exp_mask_z(h, i, z_ps, pend):
                    s_ps, col0 = pend
                    E_t = epool.tile([P, SQT], bf16, name="E_t")
                    nc.scalar.activation(E_t[:, col0:SQT], s_ps[:, col0:SQT],
                                         Exp, scale=0.125)
                    if i >= 4 * j:
                        nc.vector.tensor_tensor(
                            E_t[:, col0:col0 + P], E_t[:, col0:col0 + P],
                            maskb[:], Mult)
                    nc.tensor.matmul(
                        z_ps[0:65, col0:SQT],
                        V_all[:, i, h, :],
                        E_t[:, col0:SQT],
                        start=(i == 0), stop=(i == n_i - 1),
                        skip_group_check=True)

                def normalize(h, z_ps):
                    drow = zsb.tile([1, SQT], fp32, tag="drow", name="drow")
                    nc.vector.tensor_copy(drow[:], z_ps[64:65, :])
                    dscr = dram.tile([1, SQT], fp32, name="dscr")
                    nc.sync.dma_start(dscr[:], drow[:])
                    draw = zsb.tile([64, SQT], fp32, tag="draw", name="draw")
                    nc.sync.dma_start(draw[:], dscr[:].to_broadcast((64, SQT)))
                    dbc = zsb.tile([64, SQT], fp32, tag="dbc", name="dbc")
                    nc.vector.reciprocal_approx_fast(dbc[:], draw[:])
                    if h == 0:
                        nc.vector.tensor_tensor(zT2[0:64, :], z_ps[0:64, :],
                                                dbc[:], Mult)
                    elif h == 1:
                        z1t = zcol.tile([64, SQT], bf16, tag="z1t", name="z1t")
                        nc.vector.tensor_tensor(z1t[:], z_ps[0:64, :],
                                                dbc[:], Mult)
                        nc.sync.dma_start(zT2[64:128, :], z1t[:])
                    else:
                        nc.vector.tensor_tensor(zT3[:], z_ps[0:64, :],
                                                dbc[:], Mult)

                # pair (h0, h1) interleaved to keep PE dense
                z_ps0 = psum.tile([P, SQT], fp32, tag="z", name="z_ps0")
                z_ps1 = psum.tile([P, SQT], fp32, tag="z", name="z_ps1")
                pend0 = s_start(0, 0)
                pend1 = s_start(1, 0)
                for i in range(n_i):
                    nxt0 = s_start(0, i + 1) if i + 1 < n_i else None
                    exp_mask_z(0, i, z_ps0, pend0)
                    nxt1 = s_start(1, i + 1) if i + 1 < n_i else None
                    exp_mask_z(1, i, z_ps1, pend1)
                    pend0, pend1 = nxt0, nxt1
                normalize(0, z_ps0)
                normalize(1, z_ps1)

                # solo head
                z_ps2 = psum.tile([P, SQT], fp32, tag="z", name="z_ps2")
                pend2 = s_start(2, 0)
                for i in range(n_i):
                    nxt2 = s_start(2, i + 1) if i + 1 < n_i else None
                    exp_mask_z(2, i, z_ps2, pend2)
                    pend2 = nxt2
                normalize(2, z_ps2)

                # ---------------- output projection ----------------
                for c in range(4):
                    row = ds(SQT * j + P * c, P)
                    o1 = psum.tile([P, 512], fp32, tag="o1", name="o1")
                    nc.tensor.matmul(o1[:], zT2[:, ts(c, P)], wo2b[:, 0:512],
                                     start=True, stop=False,
                                     skip_group_check=True)
                    nc.tensor.matmul(o1[:], zT3[:, ts(c, P)], wo3b[:, 0:512],
                                     start=False, stop=True,
                                     skip_group_check=True)
                    o2 = psum.tile([P, 256], fp32, tag="o2", name="o2")
                    nc.tensor.matmul(o2[:], zT2[:, ts(c, P)], wo2b[:, 512:768],
                                     start=True, stop=False,
                                     skip_group_check=True)
                    nc.tensor.matmul(o2[:], zT3[:, ts(c, P)], wo3b[:, 512:768],
                                     start=False, stop=True,
                                     skip_group_check=True)
                    o1s = zsb.tile([P, 512], fp32, tag="o1s", name="o1s")
                    nc.vector.tensor_copy(o1s[:], o1[:])
                    o2s = zsb.tile([P, 256], fp32, tag="o2s", name="o2s")
                    nc.scalar.copy(o2s[:], o2[:])
                    nc.sync.dma_start(out[row, 0:512], o1s[:])
                    nc.sync.dma_start(out[row, 512:768], o2s[:])

    nc.compile()
    return nc


def _get_program():
    global _PROGRAM
    if _PROGRAM is None:
        _PROGRAM = _build_program()
    return _PROGRAM


def kernel(x, W_Q, W_K, W_V, W_O, b_Q, b_K, b_V, b_O):
    global LAST_RESULTS
    _install_ntff_shim()
    from concourse import bass_utils

    x = np.asarray(x, dtype=np.float32)
    W_Q = np.asarray(W_Q, dtype=np.float32)
    W_K = np.asarray(W_K, dtype=np.float32)
    W_V = np.asarray(W_V, dtype=np.float32)
    W_O = np.asarray(W_O, dtype=np.float32)
    b_Q = np.asarray(b_Q, dtype=np.float32)
    b_K = np.asarray(b_K, dtype=np.float32)
    b_V = np.asarray(b_V, dtype=np.float32)
    b_O = np.asarray(b_O, dtype=np.float32)
    assert not (np.any(b_Q) or np.any(b_K) or np.any(b_V)), \
        "kernel assumes zero QKV biases (problem spec fill=zeros)"

    nc = _get_program()

    mask = np.triu(np.ones((P, P), dtype=np.float32))
    xTs = [np.ascontiguousarray(x[b].T) for b in range(B)]

    in_maps = []
    for c in range(N_CORES):
        b, g = c // 4, c % 4
        hs = [3 * g, 3 * g + 1, 3 * g + 2]
        in_maps.append({
            "xT": xTs[b],
            "wq2": np.ascontiguousarray(
                np.concatenate([W_Q[hs[0]], W_Q[hs[1]]], axis=1)),
            "wk2": np.ascontiguousarray(
                np.concatenate([W_K[hs[0]], W_K[hs[1]]], axis=1)),
            "wqk3": np.ascontiguousarray(
                np.concatenate([W_Q[hs[2]], W_K[hs[2]]], axis=1)),
            "wv": np.ascontiguousarray(
                np.concatenate([W_V[hs[0]], W_V[hs[1]], W_V[hs[2]]], axis=1)),
            "wo2": np.ascontiguousarray(
                np.concatenate([W_O[hs[0]], W_O[hs[1]]], axis=0)),
            "wo3": np.ascontiguousarray(W_O[hs[2]]),
            "mask": mask,
        })

    res = bass_utils.run_bass_kernel_spmd(
        nc, in_maps, core_ids=list(range(N_CORES)),
        trace=bool(os.environ.get("BASS_TRACE")))
    LAST_RESULTS = res

    parts = [res.results[c]["out"] for c in range(N_CORES)]
    full = np.stack([
        parts[0] + parts[1] + parts[2] + parts[3],
        parts[4] + parts[5] + parts[6] + parts[7],
    ], axis=0)
    if np.any(b_O):
        full = full + b_O
    return full.astype(np.float32)


# revision 9
# speedup vs baseline: 1.3639x; 1.2639x over previous
"""Causal multi-head attention block on 8 trn2 NeuronCores.

Problem: B=2, S=2048, D=768, H=12, Dh=64 (fp32), causal softmax attention
with QKV projections and output projection summed over heads.

Sharding: tensor-parallel over heads x data-parallel over batch.
core c in [0,8): b = c//4, heads = {3g, 3g+1, 3g+2} with g = c%4.
Each core computes the partial output sum over its 3 heads for its batch;
the host sums the 4 partials per batch (the TP all-reduce) and stacks.

Per-core device kernel (SPMD, identical program):
  - x[b]^T fed as [768, 2048] fp32 (layout choice of the shard), cast to
    bf16 on device.
  - QK projections emit Q^T/K^T [64, 2048] per head (heads pair-stacked on
    partitions where possible); V kept [sk, 64] with a ones column appended
    so the z matmul also yields the softmax denominator D (row 64).
  - scores computed transposed: S^T[sk, sq] = K^T.T @ Q^T tiles, causal
    band only, exp on ScalarE (scale=1/8 folded in), triangular-diagonal
    masking via one [128,128] 0/1 mask multiply.
  - z^T[e, sq] accumulated over sk tiles; normalized by 1/D via a
    DMA-broadcast of the reciprocal row; output projection contracts the
    head pair jointly (K=128) plus the solo head (K=64) into shared PSUM.

Biases are all zeros per the problem spec (fill=zeros); b_O is applied on
the host if nonzero (exact). b_Q/b_K/b_V are asserted zero.
"""

import os
import sys
import types
import numpy as np

B, S, D, H, DH = 2, 2048, 768, 12, 64
N_CORES = 8
P = 128
NK = D // P      # 6 contraction chunks
NJ = S // 512    # 4 sq tiles of 512
NI = S // P      # 16 sk tiles of 128
SQT = 512

_PROGRAM = None
LAST_RESULTS = None


def _install_ntff_shim():
    """antenv.axon_hooks is missing in this image; shim it so trace=True works."""
    if "antenv.axon_hooks" in sys.modules:
        return
    try:
        from trn_agent_boot.trn_boot import _ntff_profile_via_ctypes
        m = types.ModuleType("antenv.axon_hooks")
        hook = _ntff_profile_via_ctypes("/opt/axon/libaxon_pjrt.so")
        m.get_axon_ntff_profile_hook = lambda: hook
        m.set_axon_ntff_profile_hook = lambda h: None
        sys.modules["antenv.axon_hooks"] = m
    except Exception:
        pass


def _build_program():
    import concourse.bass as bass
    import concourse.mybir as mybir
    from concourse import bacc
    from concourse.tile import TileContext
    from concourse.bass import ts, ds

    fp32 = mybir.dt.float32
    bf16 = mybir.dt.bfloat16
    Exp = mybir.ActivationFunctionType.Exp
    Mult = mybir.AluOpType.mult

    nc = bacc.Bacc("TRN2", target_bir_lowering=False, debug=False,
                   num_devices=N_CORES)

    xT = nc.dram_tensor("xT", (D, S), fp32, kind="ExternalInput")
    wq2 = nc.dram_tensor("wq2", (D, 128), fp32, kind="ExternalInput")
    wk2 = nc.dram_tensor("wk2", (D, 128), fp32, kind="ExternalInput")
    wqk3 = nc.dram_tensor("wqk3", (D, 128), fp32, kind="ExternalInput")
    wv = nc.dram_tensor("wv", (D, 192), fp32, kind="ExternalInput")
    wo2 = nc.dram_tensor("wo2", (128, D), fp32, kind="ExternalInput")
    wo3 = nc.dram_tensor("wo3", (DH, D), fp32, kind="ExternalInput")
    maskin = nc.dram_tensor("mask", (P, P), fp32, kind="ExternalInput")
    out = nc.dram_tensor("out", (S, D), fp32, kind="ExternalOutput")

    with TileContext(nc) as tc:
        with tc.tile_pool(name="work", bufs=1) as work, \
             tc.tile_pool(name="stage", bufs=2) as stage, \
             tc.tile_pool(name="epool", bufs=8) as epool, \
             tc.tile_pool(name="zsb", bufs=2) as zsb, \
             tc.tile_pool(name="zcol", bufs=2) as zcol, \
             tc.tile_pool(name="dram", bufs=2, space="DRAM") as dram, \
             tc.tile_pool(name="psum", bufs=2, space="PSUM") as psum:

            # ---------------- persistent SBUF tiles ----------------
            xTb = work.tile([P, NK, S], bf16, name="xTb")
            QT2 = work.tile([P, S], bf16, name="QT2")   # h0 rows 0:64, h1 rows 64:128
            KT2 = work.tile([P, S], bf16, name="KT2")
            QT3 = work.tile([64, S], bf16, name="QT3")
            KT3hi = work.tile([P, S], bf16, name="KT3hi")  # rows 64:128 = K3
            KT3 = work.tile([64, S], bf16, name="KT3")
            V_all = work.tile([P, NI, 3, 65], bf16, name="V_all")
            wq2b = work.tile([P, NK, 128], bf16, name="wq2b")
            wk2b = work.tile([P, NK, 128], bf16, name="wk2b")
            wqk3b = work.tile([P, NK, 128], bf16, name="wqk3b")
            wvb = work.tile([P, NK, 192], bf16, name="wvb")
            wo2b = work.tile([P, D], bf16, name="wo2b")
            wo3b = work.tile([64, D], bf16, name="wo3b")
            maskb = work.tile([P, P], bf16, name="maskb")

            # ---------------- load + cast x^T (issue DMAs first) ----------------
            xsts = []
            for k in range(NK):
                xst = stage.tile([P, S], fp32, tag="xstage", name="xst", bufs=4)
                nc.sync.dma_start(xst[:], xT[ts(k, P), :])
                xsts.append(xst)

            # ---------------- load + cast weights ----------------
            def load_cast(dst, dram_ap, shape_f32):
                st = stage.tile(shape_f32, fp32, tag="wstage", name="wstage", bufs=2)
                nc.sync.dma_start(st[:], dram_ap)
                nc.vector.tensor_copy(dst[:], st[:])

            load_cast(wq2b, wq2[:].rearrange("(o p) e -> p o e", p=P), [P, NK, 128])
            load_cast(wk2b, wk2[:].rearrange("(o p) e -> p o e", p=P), [P, NK, 128])
            load_cast(wqk3b, wqk3[:].rearrange("(o p) e -> p o e", p=P), [P, NK, 128])
            load_cast(wvb, wv[:].rearrange("(o p) e -> p o e", p=P), [P, NK, 192])

            wo2st = stage.tile([P, D], fp32, tag="wostage", name="wo2st")
            nc.sync.dma_start(wo2st[:], wo2[:])
            nc.vector.tensor_copy(wo2b[:], wo2st[:])
            wo3st = stage.tile([64, D], fp32, tag="wostage", name="wo3st")
            nc.sync.dma_start(wo3st[:], wo3[:])
            nc.vector.tensor_copy(wo3b[:], wo3st[:])
            mst = stage.tile([P, P], fp32, tag="mstage", name="mst")
            nc.sync.dma_start(mst[:], maskin[:])
            nc.vector.tensor_copy(maskb[:], mst[:])

            for k in range(NK):
                nc.vector.tensor_copy(xTb[:, k, :], xsts[k][:])

            # ---------------- QK projections ----------------
            # psum tags: "s" and "z" are shared with the attention phase.
            for st_i in range(NJ):
                sl = ts(st_i, SQT)
                q2p = psum.tile([P, SQT], fp32, tag="s", name="q2p", bufs=3)
                for k in range(NK):
                    nc.tensor.matmul(q2p[:], wq2b[:, k, :], xTb[:, k, sl],
                                     start=(k == 0), stop=(k == NK - 1))
                nc.scalar.copy(QT2[:, sl], q2p[:])

                k2p = psum.tile([P, SQT], fp32, tag="s", name="k2p", bufs=3)
                for k in range(NK):
                    nc.tensor.matmul(k2p[:], wk2b[:, k, :], xTb[:, k, sl],
                                     start=(k == 0), stop=(k == NK - 1))
                nc.scalar.copy(KT2[:, sl], k2p[:])

                qk3p = psum.tile([P, SQT], fp32, tag="s", name="qk3p", bufs=3)
                for k in range(NK):
                    nc.tensor.matmul(qk3p[:], wqk3b[:, k, :], xTb[:, k, sl],
                                     start=(k == 0), stop=(k == NK - 1))
                nc.scalar.copy(QT3[:, sl], qk3p[0:64, :])
                nc.scalar.copy(KT3hi[64:128, sl], qk3p[64:128, :])

            # K3 partition shift 64:128 -> 0:64 (DMA can cross partitions)
            nc.sync.dma_start(KT3[:, :], KT3hi[64:128, :])

            # ---------------- V projections ----------------
            for t in range(NI):
                vp = psum.tile([P, 192], fp32, tag="z", name="vp", bufs=3)
                for k in range(NK):
                    nc.tensor.matmul(vp[:], xTb[:, k, ts(t, P)], wvb[:, k, :],
                                     start=(k == 0), stop=(k == NK - 1))
                nc.scalar.copy(V_all[:, t, :, 0:64],
                               vp[:].rearrange("p (h e) -> p h e", h=3))
            nc.vector.memset(V_all[:, :, :, 64], 1.0)

            # ---------------- attention ----------------

            prev_out = []   # deferred out-projection closures from previous j

            def make_out_closures(j, zT2, zT3):
                clos = []
                for c in range(4):
                    def f1(j=j, c=c, zT2=zT2, zT3=zT3):
                        row = ds(SQT * j + P * c, P)
                        o1 = psum.tile([P, 512], fp32, tag="o1", name="o1", bufs=1)
                        nc.tensor.matmul(o1[:], zT2[:, ts(c, P)], wo2b[:, 0:512],
                                         start=True, stop=False,
                                         skip_group_check=True)
                        nc.tensor.matmul(o1[:], zT3[:, ts(c, P)], wo3b[:, 0:512],
                                         start=False, stop=True,
                                         skip_group_check=True)
                        o1s = zsb.tile([P, 512], fp32, tag="o1s", name="o1s")
                        nc.vector.tensor_copy(o1s[:], o1[:])
                        nc.sync.dma_start(out[row, 0:512], o1s[:])
                    def f2(j=j, c=c, zT2=zT2, zT3=zT3):
                        row = ds(SQT * j + P * c, P)
                        o2 = psum.tile([P, 256], fp32, tag="o2", name="o2", bufs=1)
                        nc.tensor.matmul(o2[:], zT2[:, ts(c, P)], wo2b[:, 512:768],
                                         start=True, stop=False,
                                         skip_group_check=True)
                        nc.tensor.matmul(o2[:], zT3[:, ts(c, P)], wo3b[:, 512:768],
                                         start=False, stop=True,
                                         skip_group_check=True)
                        o2s = zsb.tile([P, 256], fp32, tag="o2s", name="o2s")
                        nc.vector.tensor_copy(o2s[:], o2[:])
                        nc.sync.dma_start(out[row, 512:768], o2s[:])
                    clos.append(f1)
                    clos.append(f2)
                return clos

            for j in range(NJ):
                zT2 = zcol.tile([P, SQT], bf16, tag="zT2", name="zT2")
                zT3 = zcol.tile([64, SQT], bf16, tag="zT3", name="zT3")
                n_i = 4 * j + 4

                def s_start(h, i):
                    QTh = QT3 if h == 2 else QT2[64 * h:64 * h + 64, :]
                    KTh = KT3 if h == 2 else KT2[64 * h:64 * h + 64, :]
                    col0 = P * (i - 4 * j) if i >= 4 * j else 0
                    s_ps = psum.tile([P, SQT], fp32, tag="s", name="s_ps", bufs=3)
                    nc.tensor.matmul(
                        s_ps[:, col0:SQT],
                        KTh[:, ts(i, P)],
                        QTh[:, ds(SQT * j + col0, SQT - col0)],
                        start=True, stop=True)
                    return s_ps, col0

                def exp_mask(h, i, pend):
                    s_ps, col0 = pend
                    E_t = epool.tile([P, SQT], bf16, name="E_t")
                    nc.scalar.activation(E_t[:, col0:SQT], s_ps[:, col0:SQT],
                                         Exp, scale=0.125)
                    if i >= 4 * j:
                        nc.vector.tensor_tensor(
                            E_t[:, col0:col0 + P], E_t[:, col0:col0 + P],
                            maskb[:], Mult)
                    return E_t, col0

                def z_acc(h, i, z_ps, et):
                    E_t, col0 = et
                    nc.tensor.matmul(
                        z_ps[0:65, col0:SQT],
                        V_all[:, i, h, :],
                        E_t[:, col0:SQT],
                        start=(i == 0), stop=(i == n_i - 1),
                        skip_group_check=True)

                def normalize(h, z_ps):
                    drow = zsb.tile([1, SQT], fp32, tag="drow", name="drow")
                    nc.vector.tensor_copy(drow[:], z_ps[64:65, :])
                    dscr = dram.tile([1, SQT], fp32, name="dscr")
                    nc.sync.dma_start(dscr[:], drow[:])
                    draw = zsb.tile([64, SQT], fp32, tag="draw", name="draw")
                    nc.sync.dma_start(draw[:], dscr[:].to_broadcast((64, SQT)))
                    dbc = zsb.tile([64, SQT], fp32, tag="dbc", name="dbc")
                    nc.vector.reciprocal_approx_fast(dbc[:], draw[:])
                    if h == 0:
                        nc.vector.tensor_tensor(zT2[0:64, :], z_ps[0:64, :],
                                                dbc[:], Mult)
                    elif h == 1:
                        z1t = zcol.tile([64, SQT], bf16, tag="z1t", name="z1t")
                        nc.vector.tensor_tensor(z1t[:], z_ps[0:64, :],
                                                dbc[:], Mult)
                        nc.sync.dma_start(zT2[64:128, :], z1t[:])
                    else:
                        nc.vector.tensor_tensor(zT3[:], z_ps[0:64, :],
                                                dbc[:], Mult)

                z_pss = [psum.tile([P, SQT], fp32, tag="z", name=f"z_ps{h}",
                                   bufs=3) for h in range(3)]
                pend = [s_start(h, 0) for h in range(3)]
                for i in range(n_i):
                    ets = [exp_mask(h, i, pend[h]) for h in range(3)]
                    if i + 1 < n_i:
                        pend = [s_start(h, i + 1) for h in range(3)]
                    if prev_out:
                        prev_out.pop(0)()
                    for h in range(3):
                        z_acc(h, i, z_pss[h], ets[h])
                    if prev_out:
                        prev_out.pop(0)()
                for h in range(3):
                    normalize(h, z_pss[h])
                for f in prev_out:
                    f()
                prev_out = make_out_closures(j, zT2, zT3)

            for f in prev_out:
                f()

    nc.compile()
    return nc


def _get_program():
    global _PROGRAM
    if _PROGRAM is None:
        _PROGRAM = _build_program()
    return _PROGRAM


def kernel(x, W_Q, W_K, W_V, W_O, b_Q, b_K, b_V, b_O):
    global LAST_RESULTS
    _install_ntff_shim()
    from concourse import bass_utils

    x = np.asarray(x, dtype=np.float32)
    W_Q = np.asarray(W_Q, dtype=np.float32)
    W_K = np.asarray(W_K, dtype=np.float32)
    W_V = np.asarray(W_V, dtype=np.float32)
    W_O = np.asarray(W_O, dtype=np.float32)
    b_Q = np.asarray(b_Q, dtype=np.float32)
    b_K = np.asarray(b_K, dtype=np.float32)
    b_V = np.asarray(b_V, dtype=np.float32)
    b_O = np.asarray(b_O, dtype=np.float32)
    assert not (np.any(b_Q) or np.any(b_K) or np.any(b_V)), \
        "kernel assumes zero QKV biases (problem spec fill=zeros)"

    nc = _get_program()

    mask = np.triu(np.ones((P, P), dtype=np.float32))
    xTs = [np.ascontiguousarray(x[b].T) for b in range(B)]

    in_maps = []
    for c in range(N_CORES):
        b, g = c // 4, c % 4
        hs = [3 * g, 3 * g + 1, 3 * g + 2]
        in_maps.append({
            "xT": xTs[b],
            "wq2": np.ascontiguousarray(
                np.concatenate([W_Q[hs[0]], W_Q[hs[1]]], axis=1)),
            "wk2": np.ascontiguousarray(
                np.concatenate([W_K[hs[0]], W_K[hs[1]]], axis=1)),
            "wqk3": np.ascontiguousarray(
                np.concatenate([W_Q[hs[2]], W_K[hs[2]]], axis=1)),
            "wv": np.ascontiguousarray(
                np.concatenate([W_V[hs[0]], W_V[hs[1]], W_V[hs[2]]], axis=1)),
            "wo2": np.ascontiguousarray(
                np.concatenate([W_O[hs[0]], W_O[hs[1]]], axis=0)),
            "wo3": np.ascontiguousarray(W_O[hs[2]]),
            "mask": mask,
        })

    res = bass_utils.run_bass_kernel_spmd(
        nc, in_maps, core_ids=list(range(N_CORES)),
        trace=bool(os.environ.get("BASS_TRACE")))
    LAST_RESULTS = res

    parts = [res.results[c]["out"] for c in range(N_CORES)]
    full = np.stack([
        parts[0] + parts[1] + parts[2] + parts[3],
        parts[4] + parts[5] + parts[6] + parts[7],
    ], axis=0)
    if np.any(b_O):
        full = full + b_O
    return full.astype(np.float32)


# revision 11
# speedup vs baseline: 1.4139x; 1.0367x over previous
"""Causal multi-head attention block on 8 trn2 NeuronCores.

Problem: B=2, S=2048, D=768, H=12, Dh=64 (fp32), causal softmax attention
with QKV projections and output projection summed over heads.

Sharding: tensor-parallel over heads x data-parallel over batch.
core c in [0,8): b = c//4, heads = {3g, 3g+1, 3g+2} with g = c%4.
Each core computes the partial output sum over its 3 heads for its batch;
the host sums the 4 partials per batch (the TP all-reduce) and stacks.

Per-core device kernel (SPMD, identical program):
  - x[b]^T fed as [768, 2048] fp32 (layout choice of the shard), cast to
    bf16 on device.
  - QK projections emit Q^T/K^T [64, 2048] per head (heads pair-stacked on
    partitions where possible); V kept [sk, 64] with a ones column appended
    so the z matmul also yields the softmax denominator D (row 64).
  - scores computed transposed: S^T[sk, sq] = K^T.T @ Q^T tiles, causal
    band only, exp on ScalarE (scale=1/8 folded in), triangular-diagonal
    masking via one [128,128] 0/1 mask multiply.
  - z^T[e, sq] accumulated over sk tiles; normalized by 1/D via a
    DMA-broadcast of the reciprocal row; output projection contracts the
    head pair jointly (K=128) plus the solo head (K=64) into shared PSUM.

Biases are all zeros per the problem spec (fill=zeros); b_O is applied on
the host if nonzero (exact). b_Q/b_K/b_V are asserted zero.
"""

import os
import sys
import types
import numpy as np

B, S, D, H, DH = 2, 2048, 768, 12, 64
N_CORES = 8
P = 128
NK = D // P      # 6 contraction chunks
NJ = S // 512    # 4 sq tiles of 512
NI = S // P      # 16 sk tiles of 128
SQT = 512

_PROGRAM = None
LAST_RESULTS = None


def _install_ntff_shim():
    """antenv.axon_hooks is missing in this image; shim it so trace=True works."""
    if "antenv.axon_hooks" in sys.modules:
        return
    try:
        from trn_agent_boot.trn_boot import _ntff_profile_via_ctypes
        m = types.ModuleType("antenv.axon_hooks")
        hook = _ntff_profile_via_ctypes("/opt/axon/libaxon_pjrt.so")
        m.get_axon_ntff_profile_hook = lambda: hook
        m.set_axon_ntff_profile_hook = lambda h: None
        sys.modules["antenv.axon_hooks"] = m
    except Exception:
        pass


def _build_program():
    import concourse.bass as bass
    import concourse.mybir as mybir
    from concourse import bacc
    from concourse.tile import TileContext
    from concourse.bass import ts, ds

    fp32 = mybir.dt.float32
    bf16 = mybir.dt.bfloat16
    Exp = mybir.ActivationFunctionType.Exp
    Mult = mybir.AluOpType.mult

    nc = bacc.Bacc("TRN2", target_bir_lowering=False, debug=False,
                   num_devices=N_CORES)

    xT = nc.dram_tensor("xT", (D, S), fp32, kind="ExternalInput")
    wq2 = nc.dram_tensor("wq2", (D, 128), fp32, kind="ExternalInput")
    wk2 = nc.dram_tensor("wk2", (D, 128), fp32, kind="ExternalInput")
    wqk3 = nc.dram_tensor("wqk3", (D, 128), fp32, kind="ExternalInput")
    wv = nc.dram_tensor("wv", (D, 192), fp32, kind="ExternalInput")
    wo2 = nc.dram_tensor("wo2", (128, D), fp32, kind="ExternalInput")
    wo3 = nc.dram_tensor("wo3", (DH, D), fp32, kind="ExternalInput")
    maskin = nc.dram_tensor("mask", (P, P), fp32, kind="ExternalInput")
    out = nc.dram_tensor("out", (S, D), fp32, kind="ExternalOutput")

    with TileContext(nc) as tc:
        with tc.tile_pool(name="work", bufs=1) as work, \
             tc.tile_pool(name="stage", bufs=2) as stage, \
             tc.tile_pool(name="epool", bufs=8) as epool, \
             tc.tile_pool(name="zsb", bufs=2) as zsb, \
             tc.tile_pool(name="zcol", bufs=2) as zcol, \
             tc.tile_pool(name="dram", bufs=2, space="DRAM") as dram, \
             tc.tile_pool(name="psum", bufs=2, space="PSUM") as psum:

            # ---------------- persistent SBUF tiles ----------------
            xTb = work.tile([P, NK, S], bf16, name="xTb")
            QT2 = work.tile([P, S], bf16, name="QT2")   # h0 rows 0:64, h1 rows 64:128
            KT2 = work.tile([P, S], bf16, name="KT2")
            QT3 = work.tile([64, S], bf16, name="QT3")
            KT3hi = work.tile([P, S], bf16, name="KT3hi")  # rows 64:128 = K3
            KT3 = work.tile([64, S], bf16, name="KT3")
            V_all = work.tile([P, NI, 3, 65], bf16, name="V_all")
            wq2b = work.tile([P, NK, 128], bf16, name="wq2b")
            wk2b = work.tile([P, NK, 128], bf16, name="wk2b")
            wqk3b = work.tile([P, NK, 128], bf16, name="wqk3b")
            wvb = work.tile([P, NK, 192], bf16, name="wvb")
            wo2b = work.tile([P, D], bf16, name="wo2b")
            wo3b = work.tile([64, D], bf16, name="wo3b")
            maskb = work.tile([P, P], bf16, name="maskb")

            # ---------------- load + cast x^T (issue DMAs first) ----------------
            xsts = []
            for k in range(NK):
                xst = stage.tile([P, S], fp32, tag="xstage", name="xst", bufs=4)
                nc.sync.dma_start(xst[:], xT[ts(k, P), :])
                xsts.append(xst)

            # ---------------- load + cast weights ----------------
            def load_cast(dst, dram_ap, shape_f32):
                st = stage.tile(shape_f32, fp32, tag="wstage", name="wstage", bufs=2)
                nc.sync.dma_start(st[:], dram_ap)
                nc.vector.tensor_copy(dst[:], st[:])

            load_cast(wq2b, wq2[:].rearrange("(o p) e -> p o e", p=P), [P, NK, 128])
            load_cast(wk2b, wk2[:].rearrange("(o p) e -> p o e", p=P), [P, NK, 128])
            load_cast(wqk3b, wqk3[:].rearrange("(o p) e -> p o e", p=P), [P, NK, 128])
            load_cast(wvb, wv[:].rearrange("(o p) e -> p o e", p=P), [P, NK, 192])

            wo2st = stage.tile([P, D], fp32, tag="wostage", name="wo2st")
            nc.sync.dma_start(wo2st[:], wo2[:])
            nc.vector.tensor_copy(wo2b[:], wo2st[:])
            wo3st = stage.tile([64, D], fp32, tag="wostage", name="wo3st")
            nc.sync.dma_start(wo3st[:], wo3[:])
            nc.vector.tensor_copy(wo3b[:], wo3st[:])
            mst = stage.tile([P, P], fp32, tag="mstage", name="mst")
            nc.sync.dma_start(mst[:], maskin[:])
            nc.vector.tensor_copy(maskb[:], mst[:])

            for k in range(NK):
                nc.vector.tensor_copy(xTb[:, k, :], xsts[k][:])

            # ---------------- QK projections ----------------
            # psum tags: "s" and "z" are shared with the attention phase.
            for st_i in range(NJ):
                sl = ts(st_i, SQT)
                q2p = psum.tile([P, SQT], fp32, tag="s", name="q2p", bufs=2)
                for k in range(NK):
                    nc.tensor.matmul(q2p[:], wq2b[:, k, :], xTb[:, k, sl],
                                     start=(k == 0), stop=(k == NK - 1))
                nc.scalar.copy(QT2[:, sl], q2p[:])

                k2p = psum.tile([P, SQT], fp32, tag="s", name="k2p", bufs=2)
                for k in range(NK):
                    nc.tensor.matmul(k2p[:], wk2b[:, k, :], xTb[:, k, sl],
                                     start=(k == 0), stop=(k == NK - 1))
                nc.scalar.copy(KT2[:, sl], k2p[:])

                qk3p = psum.tile([P, SQT], fp32, tag="s", name="qk3p", bufs=2)
                for k in range(NK):
                    nc.tensor.matmul(qk3p[:], wqk3b[:, k, :], xTb[:, k, sl],
                                     start=(k == 0), stop=(k == NK - 1))
                nc.scalar.copy(QT3[:, sl], qk3p[0:64, :])
                nc.scalar.copy(KT3hi[64:128, sl], qk3p[64:128, :])

            # K3 partition shift 64:128 -> 0:64 (DMA can cross partitions)
            nc.sync.dma_start(KT3[:, :], KT3hi[64:128, :])

            # ---------------- V projections ----------------
            for t in range(NI):
                vp = psum.tile([P, 192], fp32, tag="z", name="vp", bufs=3)
                for k in range(NK):
                    nc.tensor.matmul(vp[:], xTb[:, k, ts(t, P)], wvb[:, k, :],
                                     start=(k == 0), stop=(k == NK - 1))
                nc.scalar.copy(V_all[:, t, :, 0:64],
                               vp[:].rearrange("p (h e) -> p h e", h=3))
            nc.vector.memset(V_all[:, :, :, 64], 1.0)

            # ---------------- attention ----------------

            prev_work = []  # deferred normalize + out-proj closures

            def make_closures(j, zT2, zT3, z_pss, normalize):
                clos = [lambda h=h: normalize(h, z_pss[h]) for h in range(3)]
                for c in range(4):
                    def f1(j=j, c=c, zT2=zT2, zT3=zT3):
                        row = ds(SQT * j + P * c, P)
                        o1 = psum.tile([P, 512], fp32, tag="o", name="o1", bufs=1)
                        nc.tensor.matmul(o1[:], zT2[:, ts(c, P)], wo2b[:, 0:512],
                                         start=True, stop=False,
                                         skip_group_check=True)
                        nc.tensor.matmul(o1[:], zT3[:, ts(c, P)], wo3b[:, 0:512],
                                         start=False, stop=True,
                                         skip_group_check=True)
                        o1s = zsb.tile([P, 512], fp32, tag="o1s", name="o1s")
                        nc.vector.tensor_copy(o1s[:], o1[:])
                        nc.sync.dma_start(out[row, 0:512], o1s[:])
                    def f2(j=j, c=c, zT2=zT2, zT3=zT3):
                        row = ds(SQT * j + P * c, P)
                        o2 = psum.tile([P, 512], fp32, tag="o", name="o2", bufs=1)
                        nc.tensor.matmul(o2[:, 0:256], zT2[:, ts(c, P)],
                                         wo2b[:, 512:768],
                                         start=True, stop=False,
                                         skip_group_check=True)
                        nc.tensor.matmul(o2[:, 0:256], zT3[:, ts(c, P)],
                                         wo3b[:, 512:768],
                                         start=False, stop=True,
                                         skip_group_check=True)
                        o2s = zsb.tile([P, 256], fp32, tag="o2s", name="o2s")
                        nc.vector.tensor_copy(o2s[:], o2[:, 0:256])
                        nc.sync.dma_start(out[row, 512:768], o2s[:])
                    clos.append(f1)
                    clos.append(f2)
                return clos

            for j in range(NJ):
                zT2 = zcol.tile([P, SQT], bf16, tag="zT2", name="zT2")
                zT3 = zcol.tile([64, SQT], bf16, tag="zT3", name="zT3")
                n_i = 4 * j + 4

                def col0_of(i):
                    return P * (i - 4 * j) if i >= 4 * j else 0

                def s_pair_start(i):
                    col0 = col0_of(i)
                    s_ps = psum.tile([P, 2, SQT], fp32, tag="s", name="s_ps",
                                     bufs=2)
                    for h in range(2):
                        KTh = KT2[64 * h:64 * h + 64, :]
                        QTh = QT2[64 * h:64 * h + 64, :]
                        nc.tensor.matmul(
                            s_ps[:, h, col0:SQT],
                            KTh[:, ts(i, P)],
                            QTh[:, ds(SQT * j + col0, SQT - col0)],
                            start=True, stop=True)
                    return s_ps, col0

                def s_solo_start(i):
                    col0 = col0_of(i)
                    s_ps = psum.tile([P, 2, SQT], fp32, tag="s", name="s_ps3",
                                     bufs=2)
                    nc.tensor.matmul(
                        s_ps[:, 0, col0:SQT],
                        KT3[:, ts(i, P)],
                        QT3[:, ds(SQT * j + col0, SQT - col0)],
                        start=True, stop=True)
                    return s_ps, col0

                def exp_mask_pair(i, pend):
                    s_ps, col0 = pend
                    E_t = epool.tile([P, 2, SQT], bf16, name="E_t")
                    nc.scalar.activation(E_t[:, :, col0:SQT],
                                         s_ps[:, :, col0:SQT], Exp, scale=0.125)
                    if i >= 4 * j:
                        nc.vector.tensor_tensor(
                            E_t[:, :, col0:col0 + P], E_t[:, :, col0:col0 + P],
                            maskb[:, None, :].to_broadcast((P, 2, P)), Mult)
                    return E_t, col0

                def exp_mask_solo(i, pend):
                    s_ps, col0 = pend
                    E_t = epool.tile([P, 2, SQT], bf16, name="E_t")
                    nc.scalar.activation(E_t[:, 0, col0:SQT],
                                         s_ps[:, 0, col0:SQT], Exp, scale=0.125)
                    if i >= 4 * j:
                        nc.vector.tensor_tensor(
                            E_t[:, 0, col0:col0 + P], E_t[:, 0, col0:col0 + P],
                            maskb[:], Mult)
                    return E_t, col0

                def z_acc(h, i, z_ps, E_ap, col0):
                    nc.tensor.matmul(
                        z_ps[0:65, col0:SQT],
                        V_all[:, i, h, :],
                        E_ap[:, col0:SQT],
                        start=(i == 0), stop=(i == n_i - 1),
                        skip_group_check=True)

                def normalize(h, z_ps, zT2=zT2, zT3=zT3):
                    drow = zsb.tile([1, SQT], fp32, tag="drow", name="drow")
                    nc.vector.tensor_copy(drow[:], z_ps[64:65, :])
                    dscr = dram.tile([1, SQT], fp32, name="dscr")
                    nc.sync.dma_start(dscr[:], drow[:])
                    draw = zsb.tile([64, SQT], fp32, tag="draw", name="draw")
                    nc.sync.dma_start(draw[:], dscr[:].to_broadcast((64, SQT)))
                    dbc = zsb.tile([64, SQT], fp32, tag="dbc", name="dbc")
                    nc.vector.reciprocal_approx_fast(dbc[:], draw[:])
                    if h == 0:
                        nc.vector.tensor_tensor(zT2[0:64, :], z_ps[0:64, :],
                                                dbc[:], Mult)
                    elif h == 1:
                        z1t = zcol.tile([64, SQT], bf16, tag="z1t", name="z1t")
                        nc.vector.tensor_tensor(z1t[:], z_ps[0:64, :],
                                                dbc[:], Mult)
                        nc.sync.dma_start(zT2[64:128, :], z1t[:])
                    else:
                        nc.vector.tensor_tensor(zT3[:], z_ps[0:64, :],
                                                dbc[:], Mult)

                z_pss = [psum.tile([P, SQT], fp32, tag="z", name=f"z_ps{h}",
                                   bufs=3) for h in range(3)]

                # ---- pair i-loop ----
                pend = s_pair_start(0)
                for i in range(n_i):
                    E_t, col0 = exp_mask_pair(i, pend)
                    if i + 1 < n_i:
                        pend = s_pair_start(i + 1)
                    if prev_work:
                        prev_work.pop(0)()
                    z_acc(0, i, z_pss[0], E_t[:, 0, :], col0)
                    z_acc(1, i, z_pss[1], E_t[:, 1, :], col0)
                    if prev_work:
                        prev_work.pop(0)()

                # ---- solo i-loop ----
                pend = s_solo_start(0)
                for i in range(n_i):
                    E_t, col0 = exp_mask_solo(i, pend)
                    if i + 1 < n_i:
                        pend = s_solo_start(i + 1)
                    if prev_work:
                        prev_work.pop(0)()
                    z_acc(2, i, z_pss[2], E_t[:, 0, :], col0)

                for f in prev_work:
                    f()
                prev_work = make_closures(j, zT2, zT3, z_pss, normalize)

            for f in prev_work:
                f()

    nc.compile()
    return nc


def _get_program():
    global _PROGRAM
    if _PROGRAM is None:
        _PROGRAM = _build_program()
    return _PROGRAM


def kernel(x, W_Q, W_K, W_V, W_O, b_Q, b_K, b_V, b_O):
    global LAST_RESULTS
    _install_ntff_shim()
    from concourse import bass_utils

    x = np.asarray(x, dtype=np.float32)
    W_Q = np.asarray(W_Q, dtype=np.float32)
    W_K = np.asarray(W_K, dtype=np.float32)
    W_V = np.asarray(W_V, dtype=np.float32)
    W_O = np.asarray(W_O, dtype=np.float32)
    b_Q = np.asarray(b_Q, dtype=np.float32)
    b_K = np.asarray(b_K, dtype=np.float32)
    b_V = np.asarray(b_V, dtype=np.float32)
    b_O = np.asarray(b_O, dtype=np.float32)
    assert not (np.any(b_Q) or np.any(b_K) or np.any(b_V)), \
        "kernel assumes zero QKV biases (problem spec fill=zeros)"

    nc = _get_program()

    mask = np.triu(np.ones((P, P), dtype=np.float32))
    xTs = [np.ascontiguousarray(x[b].T) for b in range(B)]

    in_maps = []
    for c in range(N_CORES):
        b, g = c // 4, c % 4
        hs = [3 * g, 3 * g + 1, 3 * g + 2]
        in_maps.append({
            "xT": xTs[b],
            "wq2": np.ascontiguousarray(
                np.concatenate([W_Q[hs[0]], W_Q[hs[1]]], axis=1)),
            "wk2": np.ascontiguousarray(
                np.concatenate([W_K[hs[0]], W_K[hs[1]]], axis=1)),
            "wqk3": np.ascontiguousarray(
                np.concatenate([W_Q[hs[2]], W_K[hs[2]]], axis=1)),
            "wv": np.ascontiguousarray(
                np.concatenate([W_V[hs[0]], W_V[hs[1]], W_V[hs[2]]], axis=1)),
            "wo2": np.ascontiguousarray(
                np.concatenate([W_O[hs[0]], W_O[hs[1]]], axis=0)),
            "wo3": np.ascontiguousarray(W_O[hs[2]]),
            "mask": mask,
        })

    res = bass_utils.run_bass_kernel_spmd(
        nc, in_maps, core_ids=list(range(N_CORES)),
        trace=bool(os.environ.get("BASS_TRACE")))
    LAST_RESULTS = res

    parts = [res.results[c]["out"] for c in range(N_CORES)]
    full = np.stack([
        parts[0] + parts[1] + parts[2] + parts[3],
        parts[4] + parts[5] + parts[6] + parts[7],
    ], axis=0)
    if np.any(b_O):
        full = full + b_O
    return full.astype(np.float32)
